# revision 1
# baseline (speedup 1.0000x reference)
"""Causal self-attention (RoPE, 16 heads) on 8 Trainium2 NeuronCores — fused.

Sharding: core s -> (batch b = s//2, head-half g = s%2). Each core computes
qkv = x_b @ w_attn[:, heads g], RoPE, causal SDPA for its 8 heads, and a
partial y_local @ w_proj[rows g] -> [T, C]. Host sums the two partials per
batch (row-parallel Megatron unshard).

v2: single fused pipeline, everything SBUF-resident in bf16:
 - x, w_attn, w_proj, cos/sin, masks converted to bf16 on host.
 - per head h: qkv piece per 512-col t-block (q,k via w-stationary matmul +
   RoPE; V directly in [t,d] layout via x-stationary matmul, no transpose),
   interleaved at block granularity with attention of head h-1 so the PE
   never waits on the ACT-engine exp chain.
 - softmax denominator: DVE accumulates exp chunks elementwise; one
   ones-matmul per (head, block) reduces across partitions + broadcasts.
 - attention of head 7 interleaves with the output projection; w_proj is
   prefetched during the head loop.
"""

import sys

sys.path.insert(0, "/opt/trn_rl_repo")

import numpy as np

import concourse.bacc as bacc
import concourse.mybir as mybir
import concourse.tile as tile

P = 128
D = 128
F32 = mybir.dt.float32
BF16 = mybir.dt.bfloat16
EXP = mybir.ActivationFunctionType.Exp

NUM_HEADS = 16
ROPE_THETA = 10000.0


def build_nc(
    T=2048,
    CIN=2048,
    HL=8,
    COUT=2048,
    *,
    w_bufs=2,
    acc_bufs=2,
    st_bufs=3,
    yps_bufs=2,
    vps_bufs=1,
    e_bufs=4,
    ds_bufs=2,
    rc_bufs=1,
    rope_bufs=2,
    qk_bufs=2,
    v_bufs=2,
    o_bufs=3,
):
    CC = CIN // P        # contraction chunks (16)
    TB = T // 512        # 512-wide t blocks (4)
    TC = T // P          # 128-wide t chunks (16)
    NB = COUT // 512     # output col blocks (4)
    SCALE = 1.0 / float(np.sqrt(D))

    nc = bacc.Bacc("TRN2", target_bir_lowering=False, debug=False)

    xT_d = nc.dram_tensor("xT", [CIN, T], BF16, kind="ExternalInput").ap()
    w_d = nc.dram_tensor("w", [HL, 3, P, CIN // P * D], BF16, kind="ExternalInput").ap()
    wp_d = nc.dram_tensor("wp", [HL * D, COUT], BF16, kind="ExternalInput").ap()
    cosT_d = nc.dram_tensor("cosT", [D, T], BF16, kind="ExternalInput").ap()
    sinT_d = nc.dram_tensor("sinT", [D, T], BF16, kind="ExternalInput").ap()
    mask_d = nc.dram_tensor("mask", [P, 896], BF16, kind="ExternalInput").ap()
    out_d = nc.dram_tensor("out", [T, COUT], BF16, kind="ExternalOutput").ap()

    with tile.TileContext(nc) as tc:
        with (
            tc.tile_pool(name="const", bufs=1) as cp,
            tc.tile_pool(name="xt", bufs=1) as xtp,
            tc.tile_pool(name="ropetab", bufs=1) as rtp,
            tc.tile_pool(name="maskp", bufs=1) as mp,
            tc.tile_pool(name="yt", bufs=1) as ytp,
            tc.tile_pool(name="wpp", bufs=1) as wpp,
            tc.tile_pool(name="w1", bufs=w_bufs) as wpool,
            tc.tile_pool(name="rope", bufs=rope_bufs) as rp,
            tc.tile_pool(name="qk", bufs=qk_bufs) as qkp,
            tc.tile_pool(name="vpool", bufs=v_bufs) as vp,
            tc.tile_pool(name="esb", bufs=e_bufs) as ep,
            tc.tile_pool(name="dsp", bufs=ds_bufs) as dsp,
            tc.tile_pool(name="rcp", bufs=rc_bufs) as rcp,
            tc.tile_pool(name="o", bufs=o_bufs) as op,
            tc.tile_pool(name="ps_acc", bufs=acc_bufs, space="PSUM") as accp,
            tc.tile_pool(name="ps_v", bufs=vps_bufs, space="PSUM") as vpsp,
            tc.tile_pool(name="ps_st", bufs=st_bufs, space="PSUM") as stp,
            tc.tile_pool(name="ps_y", bufs=yps_bufs, space="PSUM") as ypsp,
        ):
            # constants
            ones_bf = cp.tile([P, P], BF16)
            nc.vector.memset(ones_bf[:], 1.0)

            # resident tensors
            xT = xtp.tile([P, CC, T], BF16)
            cosT = rtp.tile([D, T], BF16)
            sinT = rtp.tile([D, T], BF16)
            masks = mp.tile([P, 896], BF16)
            yT = ytp.tile([P, HL, T], BF16)
            wp_sb = wpp.tile([P, HL, COUT], BF16)

            # --- startup DMAs (SP queue order matters) ---
            xr = xT_d.rearrange("(c p) t -> p c t", p=P)
            w_tiles = {}

            def load_w(h):
                w_f = wpool.tile([P, 3 * CC * D], BF16, name="w_h")
                w_h = w_f.rearrange("p (j c d) -> p j c d", j=3, c=CC)
                wr = w_d[h].rearrange("j p k -> p j k")
                if h == 0:
                    w_tiles[h] = (w_h, w_f, wr)  # DMAs interleaved below
                else:
                    nc.sync.dma_start(
                        w_f.rearrange("p (j k) -> p j k", j=3)[:], wr
                    )
                    w_tiles[h] = w_h

            load_w(0)
            w_h0, w_f0, wr0 = w_tiles[0]
            w_tiles[0] = w_h0
            KD = CC * D
            nc.sync.dma_start(xT[:, :, 0:256], xr[:, :, 0:256])
            nc.sync.dma_start(w_f0[:, 0:KD], wr0[:, 0])  # w_q0
            nc.sync.dma_start(w_f0[:, KD : 2 * KD], wr0[:, 1])  # w_k0
            nc.sync.dma_start(w_f0[:, 2 * KD :], wr0[:, 2])  # w_v0
            nc.sync.dma_start(xT[:, :, 256:512], xr[:, :, 256:512])
            nc.sync.dma_start(xT[:, :, 512:768], xr[:, :, 512:768])
            nc.sync.dma_start(xT[:, :, 768:1024], xr[:, :, 768:1024])
            nc.sync.dma_start(cosT[:], cosT_d[:])
            nc.sync.dma_start(sinT[:], sinT_d[:])
            for tb in range(2, TB):
                nc.sync.dma_start(
                    xT[:, :, tb * 512 : (tb + 1) * 512],
                    xr[:, :, tb * 512 : (tb + 1) * 512],
                )
            nc.sync.dma_start(masks[:], mask_d[:])

            qk_tiles = {}
            v_tiles = {}

            def qkv_piece(h, b):
                """q_h, k_h for t-block b (with RoPE) + V_h t-chunks 4b..4b+3."""
                w_h = w_tiles[h]
                bs = slice(b * 512, (b + 1) * 512)
                if b == 0:
                    qT_h = qkp.tile([P, T], BF16, name="qT_h")
                    kT_h = qkp.tile([P, T], BF16, name="kT_h")
                    V_h = vp.tile([P, TC, P], BF16, name="V_h")
                    qk_tiles[h] = (qT_h, kT_h)
                    v_tiles[h] = V_h
                qT_h, kT_h = qk_tiles[h]
                V_h = v_tiles[h]
                # the very first piece works in 256-col halves so compute can
                # start after half of the first xT chunk has landed
                halves = (
                    [(0, 256), (256, 512)] if (h == 0 and b == 0) else [(0, 512)]
                )
                for lo, hi in halves:
                    w_ = hi - lo
                    hs = slice(b * 512 + lo, b * 512 + hi)
                    for which, dst in ((0, qT_h), (1, kT_h)):
                        acc = accp.tile([P, 512], F32, name="acc")
                        for c in range(CC):
                            nc.tensor.matmul(
                                acc[:, 0:w_],
                                w_h[:, which, c],
                                xT[:, c, hs],
                                start=(c == 0),
                                stop=(c == CC - 1),
                            )
                        # RoPE: q' = q*cos + rot(q)*sin, rot = [-q_hi, q_lo]
                        raw = rp.tile([P, 512], F32, name="raw")
                        nc.scalar.copy(raw[:, 0:w_], acc[:, 0:w_])
                        rot = rp.tile([P, 512], F32, name="rot")
                        nc.scalar.mul(rot[0:64, 0:w_], raw[64:128, 0:w_], -1.0)
                        nc.scalar.copy(rot[64:128, 0:w_], raw[0:64, 0:w_])
                        nc.vector.tensor_mul(raw[:, 0:w_], raw[:, 0:w_], cosT[:, hs])
                        nc.vector.tensor_mul(rot[:, 0:w_], rot[:, 0:w_], sinT[:, hs])
                        nc.vector.tensor_add(dst[:, hs], raw[:, 0:w_], rot[:, 0:w_])
                    # V in [t, d] layout directly: x-stationary matmul
                    vps = vpsp.tile([P, 4, P], F32, name="vps")
                    t4s = range(lo // P, hi // P)
                    for t4 in t4s:
                        tt = 4 * b + t4
                        for c in range(CC):
                            nc.tensor.matmul(
                                vps[:, t4],
                                xT[:, c, tt * P : (tt + 1) * P],
                                w_h[:, 2, c],
                                start=(c == 0),
                                stop=(c == CC - 1),
                            )
                    nc.vector.tensor_copy(
                        V_h[:, 4 * b + t4s.start : 4 * b + t4s.stop],
                        vps[:, t4s.start : t4s.stop],
                    )

            def att_block(h, b, filler=None):
                """Causal attention for head h, q block b -> yT[:, h, block].

                filler: list of zero-arg thunks (proj tiles); one is emitted
                after each y matmul to keep the PE fed while the ACT engine
                works through the exp chain.
                """
                qT_h, kT_h = qk_tiles[h]
                V_h = v_tiles[h]
                nch = 4 * (b + 1)
                bs = slice(b * 512, (b + 1) * 512)
                yps = ypsp.tile([P, 512], F32, name="yps")
                ds = dsp.tile([P, 512], BF16, name="ds")
                es = {}
                # last two (diagonal) chunks only touch the upper 256 q
                # columns; the lower half is fully masked — skip it
                qlo = {nch - 2: 256, nch - 1: 256}
                # software-pipelined: scores(c+1) issued before y(c)
                for c in range(nch + 1):
                    if c < nch:
                        lo = qlo.get(c, 0)
                        w_ = 512 - lo
                        qs = slice(b * 512 + lo, (b + 1) * 512)
                        st = stp.tile([P, 512], F32, name="st")
                        nc.tensor.matmul(
                            st[:, 0:w_],
                            kT_h[:, c * P : (c + 1) * P],
                            qT_h[:, qs],
                            start=True,
                            stop=True,
                        )
                        e_sb = ep.tile([P, 512], BF16, name="e_sb")
                        nc.scalar.activation(
                            e_sb[:, 0:w_], st[:, 0:w_], EXP, scale=SCALE
                        )
                        j = c - (nch - 4)
                        if j >= 0:
                            nc.vector.tensor_mul(
                                e_sb[:, 0:w_], e_sb[:, 0:w_],
                                masks[:, 384 - 128 * j + lo : 896 - 128 * j],
                            )
                        if c == 0:
                            nc.vector.tensor_copy(ds[:], e_sb[:])
                        else:
                            nc.vector.tensor_add(
                                ds[:, lo:512], ds[:, lo:512], e_sb[:, 0:w_]
                            )
                        es[c] = e_sb
                    if c >= 1:
                        cc = c - 1
                        lo = qlo.get(cc, 0)
                        nc.tensor.matmul(
                            yps[:, lo:512],
                            V_h[:, cc],
                            es.pop(cc)[:, 0 : 512 - lo],
                            start=(cc == 0),
                            stop=(cc == nch - 1),
                        )
                        if filler:
                            filler.pop(0)()
                # denominator: partition-sum + broadcast in one matmul
                bcs = stp.tile([P, 512], F32, name="st")
                nc.tensor.matmul(bcs[:], ones_bf[:], ds[:], start=True, stop=True)
                recip = rcp.tile([P, 512], BF16, name="recip")
                with nc.allow_low_precision(reason="bf16 softmax recip"):
                    nc.vector.reciprocal(recip[:], bcs[:])
                nc.vector.tensor_mul(yT[:, h, bs], yps[:], recip[:])

            def proj_tile(tt, nb):
                """One out tile: out[tt, nb] = sum_h yT[:,h,tt].T @ wp."""
                # rotate across 3 PSUM banks (2 in accp + the idle
                # vps bank) so copy-out latency never stalls the PE
                g = tt * NB + nb
                pool = vpsp if g % 3 == 2 else accp
                name = "vps" if g % 3 == 2 else "acc"
                ps3 = pool.tile([P, 512], F32, name=name)
                for hh in range(HL):
                    nc.tensor.matmul(
                        ps3[:],
                        yT[:, hh, tt * P : (tt + 1) * P],
                        wp_sb[:, hh, nb * 512 : (nb + 1) * 512],
                        start=(hh == 0),
                        stop=(hh == HL - 1),
                    )
                o_sb = op.tile([P, 512], BF16, name="o_sb")
                dst = out_d[
                    tt * P : (tt + 1) * P, nb * 512 : (nb + 1) * 512
                ]
                nc.scalar.copy(o_sb[:], ps3[:])
                nc.sync.dma_start(dst, o_sb[:])

            def proj_thunks(b):
                return [
                    (lambda tt=4 * b + t4, nb=nb: proj_tile(tt, nb))
                    for t4 in range(4)
                    for nb in range(NB)
                ]

            # --- fused pipeline ---
            for h in range(HL):
                if h + 1 < HL:
                    load_w(h + 1)  # prefetch next head's weights
                if h == 2:
                    nc.sync.dma_start(
                        wp_sb[:],
                        wp_d.rearrange("(h p) n -> p h n", p=P),
                    )
                for b in range(TB):
                    qkv_piece(h, b)
                    if h >= 1:
                        att_block(h - 1, b)
                    # last head: pull its attention forward one block so
                    # only one att block remains after the qkv rows
                    if h == HL - 1 and b >= 1:
                        att_block(HL - 1, b - 1)
            # tail: last att block with ready proj tiles as PE filler
            avail = []
            for b in range(TB - 1):
                avail.extend(proj_thunks(b))
            att_block(HL - 1, TB - 1, filler=avail)
            avail.extend(proj_thunks(TB - 1))
            for t in avail:
                t()

    nc.compile()
    return nc


def _rope_tables_T(T, head_dim):
    half = head_dim // 2
    inv_freq = 1.0 / (ROPE_THETA ** (np.arange(0, half, dtype=np.float64) / half))
    ang = np.arange(T, dtype=np.float64)[:, None] * inv_freq[None, :]  # [T, half]
    cos = np.concatenate([np.cos(ang), np.cos(ang)], axis=-1)  # [T, D]
    sin = np.concatenate([np.sin(ang), np.sin(ang)], axis=-1)
    return (
        np.ascontiguousarray(cos.T.astype(np.float32)),
        np.ascontiguousarray(sin.T.astype(np.float32)),
    )


def _make_masks():
    # masks[p, o] = (o >= p + 384); mask_j = masks[:, 384-128j : 896-128j]
    o = np.arange(896)[None, :]
    p = np.arange(P)[:, None]
    return (o >= p + 384).astype(np.float32)


_NC_CACHE = {}


def _get_nc(T, CIN, HL, COUT):
    key = (T, CIN, HL, COUT)
    if key not in _NC_CACHE:
        _NC_CACHE[key] = build_nc(T, CIN, HL, COUT)
    return _NC_CACHE[key]


def make_in_maps(x, w_attn, w_proj):
    import ml_dtypes

    bf16 = ml_dtypes.bfloat16
    x = np.asarray(x)
    w_attn = np.asarray(w_attn)
    w_proj = np.asarray(w_proj)
    B, T, C = x.shape
    HL = NUM_HEADS // 2  # 8 heads per core
    CL = HL * D  # 1024

    cosT, sinT = _rope_tables_T(T, D)
    cosT = cosT.astype(bf16)
    sinT = sinT.astype(bf16)
    masks = _make_masks().astype(bf16)

    in_maps = []
    for s in range(8):
        b, g = s // 2, s % 2
        # w layout [HL, 3, P, CC*D]: each (head, q/k/v) slab stored
        # partition-major so DMA descriptors are 4KB-contiguous
        CC = C // P
        qkv_cols = [
            w_attn[:, g * CL : (g + 1) * CL],
            w_attn[:, C + g * CL : C + (g + 1) * CL],
            w_attn[:, 2 * C + g * CL : 2 * C + (g + 1) * CL],
        ]
        w_shard = np.empty((HL, 3, P, CC * D), dtype=np.float32)
        for j, wj in enumerate(qkv_cols):
            # wj: [C, HL*D] -> per head slab [P, CC*D]
            s = wj.reshape(CC, P, HL, D).transpose(2, 1, 0, 3)  # [HL, P, CC, D]
            w_shard[:, j] = s.reshape(HL, P, CC * D)
        in_maps.append(
            {
                "xT": np.ascontiguousarray(x[b].T).astype(bf16),
                "w": np.ascontiguousarray(w_shard).astype(bf16),
                "wp": np.ascontiguousarray(
                    w_proj[g * CL : (g + 1) * CL, :]
                ).astype(bf16),
                "cosT": cosT,
                "sinT": sinT,
                "mask": masks,
            }
        )
    return in_maps


def combine(results, x_shape):
    B, T, C = x_shape
    out = np.empty((B, T, C), dtype=np.float32)
    for b in range(B):
        out[b] = results[2 * b]["out"].astype(np.float32) + results[
            2 * b + 1
        ]["out"].astype(np.float32)
    return out


def kernel(x, w_attn, w_proj):
    from concourse.bass_utils import run_bass_kernel_spmd

    x = np.asarray(x)
    B, T, C = x.shape  # 4, 2048, 2048
    HL = NUM_HEADS // 2

    nc = _get_nc(T, C, HL, C)
    in_maps = make_in_maps(x, w_attn, w_proj)
    res = run_bass_kernel_spmd(nc, in_maps, list(range(8)))
    return combine(res.results, (B, T, C))



# revision 2
# speedup vs baseline: 1.2715x; 1.2715x over previous
"""Causal self-attention (RoPE, 16 heads) on 8 Trainium2 NeuronCores — v3 fp8.

Sharding: core s -> (batch b = s//2, head-half g = s%2). Each core computes
qkv = x_b @ w_attn[:, heads g], RoPE, causal SDPA for its 8 heads, and a
partial y_local @ w_proj[rows g] -> [T, C]. Host sums the two partials per
batch (row-parallel Megatron unshard).

v3: fp8e4m3 DoubleRow matmuls with error compensation.
 - DoubleRow fp8 matmuls process two K-tiles per instruction at 0.5
   cycles/row (2x bf16 FLOPs, 4x when both slots carry fresh data).
 - qkv + proj gemms: 3-term compensated products over chunk pairs
   (a8@b8 + ar8@b8 + a8@br8 with (value, residual) fp8 pairs) = bf16-level
   accuracy at 0.75x the bf16 cycle cost. x/w/wp residuals from host.
 - scores: DR with stationary (k8, kr8) compensation pair x stride-0
   duplicated q8 moving (q8 quantization noise rides, ~1e-2).
 - attention weights e8 = exp(s) in fp8 (noise ~1.4e-2, partially
   cancelled by the shared-denominator normalization).
 - causal mask: additive -57600 matmuls accumulated straight into the
   score PSUM group (identity x mask-pattern), no vector masking.
 - softmax denominator: ones-DR matmul accumulation over e8 pairs
   (column-sum broadcast to all partitions), recip on DVE.
 - RoPE: rotate on Pool (partition-shifted copies), combine muls split
   DVE/Pool, raw PSUM->SBUF copies on ACT.
"""

import sys

sys.path.insert(0, "/opt/trn_rl_repo")

import numpy as np

import concourse.bacc as bacc
import concourse.mybir as mybir
import concourse.tile as tile

P = 128
D = 128
F32 = mybir.dt.float32
BF16 = mybir.dt.bfloat16
FP8 = mybir.dt.float8e4
EXP = mybir.ActivationFunctionType.Exp
Dd = mybir.MatmulPerfMode.DoubleRow
MUL = mybir.AluOpType.mult
SUB = mybir.AluOpType.subtract

NUM_HEADS = 16
ROPE_THETA = 10000.0

# fp8 scales
SX = 16.0      # x8 = x * 16
SW = 512.0     # w8 = w * 512
SQK = 16.0     # q8/k8 post-rope
SV = 8.0       # v8
SY = 8.0       # y8
SWP = 512.0    # wp8
MBIG = 240.0   # mask magnitude (id2 * madd -> -57600 in score psum)


def build_nc(
    T=2048,
    CIN=2048,
    HL=8,
    COUT=2048,
    *,
    w_bufs=2,
    acc_bufs=2,
    st_bufs=3,
    e_bufs=3,
    qk_bufs=2,
    v_bufs=2,
    o_bufs=3,
    rope_bufs=3,
    tmp_bufs=2,
):
    CC = CIN // P        # contraction chunks (16)
    CP = CC // 2         # contraction chunk pairs (8)
    TB = T // 512        # 512-wide t blocks (4)
    TC = T // P          # 128-wide t chunks (16)
    NB = COUT // 512     # output col blocks (4)
    SCALE = 1.0 / float(np.sqrt(D))
    EXP_SCALE = SCALE / (SQK * SQK)     # score psum is q8*k8 = s*256
    V_SCALE = SV / (SX * SW)            # v psum is x8*w8 = v*8192
    O_SCALE = 1.0 / (SY * SWP)          # proj psum is y8*wp8 = out*4096

    nc = bacc.Bacc("TRN2", target_bir_lowering=False, debug=False)

    x8_d = nc.dram_tensor("x8", [CIN, T], FP8, kind="ExternalInput").ap()
    xr8_d = nc.dram_tensor("xr8", [CIN, T], FP8, kind="ExternalInput").ap()
    w8_d = nc.dram_tensor("w8", [HL, 3, P, CC * D], FP8, kind="ExternalInput").ap()
    wr8_d = nc.dram_tensor("wr8", [HL, 3, P, CC * D], FP8, kind="ExternalInput").ap()
    wp8_d = nc.dram_tensor("wp8", [HL * D, COUT], FP8, kind="ExternalInput").ap()
    wpr8_d = nc.dram_tensor("wpr8", [HL * D, COUT], FP8, kind="ExternalInput").ap()
    cosT_d = nc.dram_tensor("cosT", [D, T], BF16, kind="ExternalInput").ap()
    sinT_d = nc.dram_tensor("sinT", [D, T], BF16, kind="ExternalInput").ap()
    madd_d = nc.dram_tensor("madd", [P, 2, 896], FP8, kind="ExternalInput").ap()
    id2_d = nc.dram_tensor("id2", [P, 2, P], FP8, kind="ExternalInput").ap()
    out_d = nc.dram_tensor("out", [T, COUT], BF16, kind="ExternalOutput").ap()

    with tile.TileContext(nc) as tc:
        with (
            tc.tile_pool(name="const", bufs=1) as cp,
            tc.tile_pool(name="xt", bufs=1) as xtp,
            tc.tile_pool(name="ropetab", bufs=1) as rtp,
            tc.tile_pool(name="maskp", bufs=1) as mp,
            tc.tile_pool(name="yt", bufs=1) as ytp,
            tc.tile_pool(name="wpp", bufs=1) as wpp,
            tc.tile_pool(name="w1", bufs=w_bufs) as wpool,
            tc.tile_pool(name="rope", bufs=rope_bufs) as rp,
            tc.tile_pool(name="qk", bufs=qk_bufs) as qkp,
            tc.tile_pool(name="vpool", bufs=v_bufs) as vp,
            tc.tile_pool(name="esb", bufs=e_bufs) as ep,
            tc.tile_pool(name="tmpp", bufs=tmp_bufs) as tp,
            tc.tile_pool(name="o", bufs=o_bufs) as op,
            tc.tile_pool(name="ps_acc", bufs=acc_bufs, space="PSUM") as accp,
            tc.tile_pool(name="ps_v", bufs=1, space="PSUM") as vpsp,
            tc.tile_pool(name="ps_st", bufs=st_bufs, space="PSUM") as stp,
            tc.tile_pool(name="ps_d", bufs=1, space="PSUM") as dpsp,
            tc.tile_pool(name="ps_y", bufs=1, space="PSUM") as ypsp,
        ):
            # constants
            ones8 = cp.tile([P, 2, P], FP8)
            nc.vector.memset(ones8[:], 1.0)

            # resident tensors
            x8 = xtp.tile([P, CC, T], FP8)
            xr8 = xtp.tile([P, CC, T], FP8)
            cosT = rtp.tile([D, T], BF16)
            sinT = rtp.tile([D, T], BF16)
            madd = mp.tile([P, 2, 896], FP8)
            id2 = mp.tile([P, 2, P], FP8)
            y8 = ytp.tile([P, HL, T], FP8)
            yr8 = ytp.tile([P, HL, T], FP8)
            wp8 = wpp.tile([P, HL, COUT], FP8)
            wpr8 = wpp.tile([P, HL, COUT], FP8)

            # --- startup DMAs (SP queue order matters) ---
            x8r = x8_d.rearrange("(c p) t -> p c t", p=P)
            xr8r = xr8_d.rearrange("(c p) t -> p c t", p=P)
            w_tiles = {}

            def load_w(h, interleave=False):
                w8f = wpool.tile([P, 3 * CC * D], FP8, name="w8_h")
                wr8f = wpool.tile([P, 3 * CC * D], FP8, name="wr8_h")
                w8h = w8f.rearrange("p (j c d) -> p j c d", j=3, c=CC)
                wr8h = wr8f.rearrange("p (j c d) -> p j c d", j=3, c=CC)
                wv8 = w8_d[h].rearrange("j p k -> p j k")
                wvr8 = wr8_d[h].rearrange("j p k -> p j k")
                if interleave:
                    return (w8h, wr8h, w8f, wr8f, wv8, wvr8)
                nc.sync.dma_start(
                    w8f.rearrange("p (j k) -> p j k", j=3)[:], wv8
                )
                nc.sync.dma_start(
                    wr8f.rearrange("p (j k) -> p j k", j=3)[:], wvr8
                )
                w_tiles[h] = (w8h, wr8h)

            w8h0, wr8h0, w8f0, wr8f0, wv80, wvr80 = load_w(0, interleave=True)
            w_tiles[0] = (w8h0, wr8h0)
            KD = CC * D
            nc.sync.dma_start(x8[:, :, 0:256], x8r[:, :, 0:256])
            nc.sync.dma_start(w8f0[:, 0:KD], wv80[:, 0])
            nc.sync.dma_start(w8f0[:, KD : 2 * KD], wv80[:, 1])
            nc.sync.dma_start(wr8f0[:, 0:KD], wvr80[:, 0])
            nc.sync.dma_start(wr8f0[:, KD : 2 * KD], wvr80[:, 1])
            nc.sync.dma_start(xr8[:, :, 0:256], xr8r[:, :, 0:256])
            nc.sync.dma_start(w8f0[:, 2 * KD :], wv80[:, 2])
            nc.sync.dma_start(wr8f0[:, 2 * KD :], wvr80[:, 2])
            nc.sync.dma_start(x8[:, :, 256:512], x8r[:, :, 256:512])
            nc.sync.dma_start(xr8[:, :, 256:512], xr8r[:, :, 256:512])
            nc.sync.dma_start(cosT[:], cosT_d[:])
            nc.sync.dma_start(sinT[:], sinT_d[:])
            nc.sync.dma_start(madd[:], madd_d[:])
            nc.sync.dma_start(id2[:], id2_d[:])
            for tb in range(1, TB):
                s = slice(tb * 512, (tb + 1) * 512)
                nc.sync.dma_start(x8[:, :, s], x8r[:, :, s])
                nc.sync.dma_start(xr8[:, :, s], xr8r[:, :, s])

            qk_tiles = {}
            v_tiles = {}

            def rope_combine(j, acc, h, b, lo, hi):
                """acc (psum, q*8192) -> q8 / (k8, kr8) fp8 at scale 16."""
                w_ = hi - lo
                hs = slice(b * 512 + lo, b * 512 + hi)
                q8t, k82 = qk_tiles[h]
                raw = rp.tile([P, 512], BF16, name="raw")
                nc.scalar.copy(raw[:, 0:w_], acc[:, 0:w_])
                rot = rp.tile([P, 512], BF16, name="rot")
                nc.gpsimd.tensor_scalar_mul(
                    rot[0:64, 0:w_], raw[64:128, 0:w_], -1.0
                )
                nc.gpsimd.tensor_copy(rot[64:128, 0:w_], raw[0:64, 0:w_])
                qc = rp.tile([P, 512], BF16, name="qc")
                # cos/sin tables pre-scaled by SQK/(SX*SW) on host
                nc.vector.tensor_mul(qc[:, 0:w_], raw[:, 0:w_], cosT[:, hs])
                qs = rp.tile([P, 512], BF16, name="qs")
                nc.gpsimd.tensor_mul(qs[:, 0:w_], rot[:, 0:w_], sinT[:, hs])
                if j == 0:
                    nc.vector.tensor_add(q8t[:, 0, hs], qc[:, 0:w_], qs[:, 0:w_])
                else:
                    kbf = rp.tile([P, 512], BF16, name="kbf")
                    nc.vector.tensor_add(kbf[:, 0:w_], qc[:, 0:w_], qs[:, 0:w_])
                    nc.gpsimd.tensor_copy(k82[:, 0, hs], kbf[:, 0:w_])
                    nc.gpsimd.tensor_tensor(
                        k82[:, 1, hs], kbf[:, 0:w_], k82[:, 0, hs], SUB
                    )

            def qkv_piece(h, b):
                """q8/k82 for t-block b (with RoPE) + v8/vr8 t-chunks 4b..4b+3."""
                w8h, wr8h = w_tiles[h]
                if b == 0:
                    q8t = qkp.tile([P, 1, T], FP8, name="q8t")
                    k82 = qkp.tile([P, 2, T], FP8, name="k82")
                    v8 = vp.tile([P, TC, D], FP8, name="v8")
                    vr8 = vp.tile([P, TC, D], FP8, name="vr8")
                    qk_tiles[h] = (q8t, k82)
                    v_tiles[h] = (v8, vr8)
                v8, vr8 = v_tiles[h]
                halves = (
                    [(0, 256), (256, 512)] if (h == 0 and b == 0) else [(0, 512)]
                )
                for lo, hi in halves:
                    w_ = hi - lo
                    hs = slice(b * 512 + lo, b * 512 + hi)
                    for j in range(2):  # q, k gemms: 3-term over chunk pairs
                        acc = accp.tile([P, 512], F32, name="acc")
                        for cpi in range(CP):
                            c = 2 * cpi
                            cs = slice(c, c + 2)
                            nc.tensor.matmul(
                                acc[:, 0:w_],
                                w8h[:, j, cs],
                                x8[:, cs, hs],
                                start=(cpi == 0),
                                stop=False,
                                perf_mode=Dd,
                            )
                            nc.tensor.matmul(
                                acc[:, 0:w_],
                                wr8h[:, j, cs],
                                x8[:, cs, hs],
                                start=False,
                                stop=False,
                                perf_mode=Dd,
                            )
                            nc.tensor.matmul(
                                acc[:, 0:w_],
                                w8h[:, j, cs],
                                xr8[:, cs, hs],
                                start=False,
                                stop=(cpi == CP - 1),
                                perf_mode=Dd,
                            )
                        rope_combine(j, acc, h, b, lo, hi)
                    # V in [t, d] layout: x-stationary 3-term
                    vps = vpsp.tile([P, 4, P], F32, name="vps")
                    t4s = range(lo // P, hi // P)
                    for t4 in t4s:
                        tt = 4 * b + t4
                        ts_ = slice(tt * P, (tt + 1) * P)
                        for cpi in range(CP):
                            cs = slice(2 * cpi, 2 * cpi + 2)
                            nc.tensor.matmul(
                                vps[:, t4],
                                x8[:, cs, ts_],
                                w8h[:, 2, cs],
                                start=(cpi == 0),
                                stop=False,
                                perf_mode=Dd,
                            )
                            nc.tensor.matmul(
                                vps[:, t4],
                                xr8[:, cs, ts_],
                                w8h[:, 2, cs],
                                start=False,
                                stop=False,
                                perf_mode=Dd,
                            )
                            nc.tensor.matmul(
                                vps[:, t4],
                                x8[:, cs, ts_],
                                wr8h[:, 2, cs],
                                start=False,
                                stop=(cpi == CP - 1),
                                perf_mode=Dd,
                            )
                    vsl = slice(4 * b + t4s.start, 4 * b + t4s.stop)
                    nc.scalar.mul(
                        v8[:, vsl, :], vps[:, t4s.start : t4s.stop], V_SCALE
                    )
                    nc.vector.scalar_tensor_tensor(
                        vr8[:, vsl, :],
                        vps[:, t4s.start : t4s.stop],
                        V_SCALE,
                        v8[:, vsl, :],
                        MUL,
                        SUB,
                    )

            def att_block(h, b, filler=None):
                """Causal attention for head h, q block b -> y8/yr8[:, h, block]."""
                q8t, k82 = qk_tiles[h]
                v8, vr8 = v_tiles[h]
                nch = 4 * (b + 1)
                npair = nch // 2
                bs = slice(b * 512, (b + 1) * 512)
                yps = ypsp.tile([P, 512], F32, name="yps")
                dps = dpsp.tile([P, 512], F32, name="dps")
                es = {}
                qlo = {nch - 2: 256, nch - 1: 256}
                # software-pipelined: scores(c+1) issued before y(pair of c)
                for c in range(nch + 2):
                    if c < nch:
                        lo = qlo.get(c, 0)
                        w_ = 512 - lo
                        qs_ = slice(b * 512 + lo, (b + 1) * 512)
                        st = stp.tile([P, 512], F32, name="st")
                        j = c - (nch - 4)
                        nc.tensor.matmul(
                            st[:, 0:w_],
                            k82[:, :, c * P : (c + 1) * P],
                            q8t[:, 0:1, qs_].broadcast_to((P, 2, w_)),
                            start=True,
                            stop=(j < 0),
                            perf_mode=Dd,
                        )
                        if j >= 0:
                            # additive causal mask into the same psum group
                            wj = 128 * (j + 1) - lo
                            ms = 384 - 128 * j + lo
                            nc.tensor.matmul(
                                st[:, 0:wj],
                                id2[:],
                                madd[:, :, ms : ms + wj],
                                start=False,
                                stop=True,
                                perf_mode=Dd,
                            )
                        if c % 2 == 0:
                            e2 = ep.tile([P, 2, 512], FP8, name="e2")
                            es[c // 2] = e2
                        e2 = es[c // 2]
                        nc.scalar.activation(
                            e2[:, c % 2, 0:w_], st[:, 0:w_], EXP, scale=EXP_SCALE
                        )
                    if c >= 2 and c % 2 == 0:
                        p = c // 2 - 1
                        lo = qlo.get(2 * p, 0)
                        w_ = 512 - lo
                        e2p = es.pop(p)
                        vs = slice(2 * p, 2 * p + 2)
                        nc.tensor.matmul(
                            yps[:, lo:512],
                            v8[:, vs, :],
                            e2p[:, :, 0:w_],
                            start=(p == 0),
                            stop=False,
                            perf_mode=Dd,
                        )
                        nc.tensor.matmul(
                            yps[:, lo:512],
                            vr8[:, vs, :],
                            e2p[:, :, 0:w_],
                            start=False,
                            stop=(p == npair - 1),
                            perf_mode=Dd,
                        )
                        nc.tensor.matmul(
                            dps[:, lo:512],
                            ones8[:],
                            e2p[:, :, 0:w_],
                            start=(p == 0),
                            stop=(p == npair - 1),
                            perf_mode=Dd,
                        )
                        if filler:
                            filler.pop(0)()
                recip = tp.tile([P, 512], BF16, name="recip")
                with nc.allow_low_precision(reason="bf16 softmax recip"):
                    nc.vector.reciprocal(recip[:], dps[:])
                tmp = tp.tile([P, 512], BF16, name="tmp")
                nc.vector.tensor_mul(tmp[:], yps[:], recip[:])
                nc.gpsimd.tensor_copy(y8[:, h, bs], tmp[:])
                nc.gpsimd.tensor_tensor(yr8[:, h, bs], tmp[:], y8[:, h, bs], SUB)

            def proj_tile(tt, nb):
                """One out tile: out[tt, nb] = sum_h y[:,h,tt].T @ wp (3-term)."""
                g = tt * NB + nb
                pool = vpsp if g % 3 == 2 else accp
                name = "vps" if g % 3 == 2 else "acc"
                ps3 = pool.tile([P, 512], F32, name=name)
                ts_ = slice(tt * P, (tt + 1) * P)
                ns = slice(nb * 512, (nb + 1) * 512)
                for hp in range(HL // 2):
                    hsl = slice(2 * hp, 2 * hp + 2)
                    nc.tensor.matmul(
                        ps3[:],
                        y8[:, hsl, ts_],
                        wp8[:, hsl, ns],
                        start=(hp == 0),
                        stop=False,
                        perf_mode=Dd,
                    )
                    nc.tensor.matmul(
                        ps3[:],
                        yr8[:, hsl, ts_],
                        wp8[:, hsl, ns],
                        start=False,
                        stop=False,
                        perf_mode=Dd,
                    )
                    nc.tensor.matmul(
                        ps3[:],
                        y8[:, hsl, ts_],
                        wpr8[:, hsl, ns],
                        start=False,
                        stop=(hp == HL // 2 - 1),
                        perf_mode=Dd,
                    )
                o_sb = op.tile([P, 512], BF16, name="o_sb")
                nc.scalar.mul(o_sb[:], ps3[:], O_SCALE)
                nc.sync.dma_start(out_d[ts_, ns], o_sb[:])

            def proj_thunks(b):
                return [
                    (lambda tt=4 * b + t4, nb=nb: proj_tile(tt, nb))
                    for t4 in range(4)
                    for nb in range(NB)
                ]

            # --- fused pipeline ---
            for h in range(HL):
                if h + 1 < HL:
                    load_w(h + 1)  # prefetch next head's weights
                if h == 2:
                    nc.sync.dma_start(
                        wp8[:], wp8_d.rearrange("(h p) n -> p h n", p=P)
                    )
                    nc.sync.dma_start(
                        wpr8[:], wpr8_d.rearrange("(h p) n -> p h n", p=P)
                    )
                for b in range(TB):
                    qkv_piece(h, b)
                    if h >= 1:
                        att_block(h - 1, b)
                    if h == HL - 1 and b >= 1:
                        att_block(HL - 1, b - 1)
            avail = []
            for b in range(TB - 1):
                avail.extend(proj_thunks(b))
            att_block(HL - 1, TB - 1, filler=avail)
            avail.extend(proj_thunks(TB - 1))
            for t in avail:
                t()

    nc.compile()
    return nc


def _rope_tables_T(T, head_dim):
    half = head_dim // 2
    inv_freq = 1.0 / (ROPE_THETA ** (np.arange(0, half, dtype=np.float64) / half))
    ang = np.arange(T, dtype=np.float64)[:, None] * inv_freq[None, :]  # [T, half]
    cos = np.concatenate([np.cos(ang), np.cos(ang)], axis=-1)  # [T, D]
    sin = np.concatenate([np.sin(ang), np.sin(ang)], axis=-1)
    return (
        np.ascontiguousarray(cos.T.astype(np.float32)),
        np.ascontiguousarray(sin.T.astype(np.float32)),
    )


_NC_CACHE = {}


def _get_nc(T, CIN, HL, COUT):
    key = (T, CIN, HL, COUT)
    if key not in _NC_CACHE:
        _NC_CACHE[key] = build_nc(T, CIN, HL, COUT)
    return _NC_CACHE[key]


def make_in_maps(x, w_attn, w_proj):
    import ml_dtypes

    f8 = ml_dtypes.float8_e4m3
    bf16 = ml_dtypes.bfloat16

    def q8pair(a, s):
        v8 = (np.asarray(a, np.float32) * s).astype(f8)
        r8 = (np.asarray(a, np.float32) * s - v8.astype(np.float32)).astype(f8)
        return v8, r8

    x = np.asarray(x)
    w_attn = np.asarray(w_attn)
    w_proj = np.asarray(w_proj)
    B, T, C = x.shape
    HL = NUM_HEADS // 2  # 8 heads per core
    CL = HL * D  # 1024
    CC = C // P

    cosT, sinT = _rope_tables_T(T, D)
    # fold SQK/(SX*SW) into the tables: psum is q*8192, out target q*16
    tab_scale = SQK / (SX * SW)
    cosT = (cosT * tab_scale).astype(bf16)
    sinT = (sinT * tab_scale).astype(bf16)

    # additive causal mask pattern: madd[p, 0, u] = -MBIG if u < p+384
    u = np.arange(896)[None, :]
    pp = np.arange(P)[:, None]
    madd = np.zeros((P, 2, 896), np.float32)
    madd[:, 0, :] = np.where(u < pp + 384, -MBIG, 0.0)
    madd = madd.astype(f8)
    id2 = np.zeros((P, 2, P), np.float32)
    id2[:, 0, :] = np.eye(P) * MBIG
    id2 = id2.astype(f8)

    wp_shards = []
    w_shards = []
    for g in range(2):
        qkv_cols = [
            w_attn[:, g * CL : (g + 1) * CL],
            w_attn[:, C + g * CL : C + (g + 1) * CL],
            w_attn[:, 2 * C + g * CL : 2 * C + (g + 1) * CL],
        ]
        w_shard = np.empty((HL, 3, P, CC * D), dtype=np.float32)
        for j, wj in enumerate(qkv_cols):
            s = wj.reshape(CC, P, HL, D).transpose(2, 1, 0, 3)  # [HL, P, CC, D]
            w_shard[:, j] = s.reshape(HL, P, CC * D)
        w_shards.append(q8pair(w_shard, SW))
        wp_shards.append(
            q8pair(np.ascontiguousarray(w_proj[g * CL : (g + 1) * CL, :]), SWP)
        )

    in_maps = []
    for s in range(8):
        b, g = s // 2, s % 2
        xT = np.ascontiguousarray(x[b].T)
        x8, xr8 = q8pair(xT, SX)
        w8, wr8 = w_shards[g]
        wp8, wpr8 = wp_shards[g]
        in_maps.append(
            {
                "x8": x8,
                "xr8": xr8,
                "w8": w8,
                "wr8": wr8,
                "wp8": wp8,
                "wpr8": wpr8,
                "cosT": cosT,
                "sinT": sinT,
                "madd": madd,
                "id2": id2,
            }
        )
    return in_maps


def combine(results, x_shape):
    B, T, C = x_shape
    out = np.empty((B, T, C), dtype=np.float32)
    for b in range(B):
        out[b] = results[2 * b]["out"].astype(np.float32) + results[
            2 * b + 1
        ]["out"].astype(np.float32)
    return out


def kernel(x, w_attn, w_proj):
    from concourse.bass_utils import run_bass_kernel_spmd

    x = np.asarray(x)
    B, T, C = x.shape  # 4, 2048, 2048
    HL = NUM_HEADS // 2

    nc = _get_nc(T, C, HL, C)
    in_maps = make_in_maps(x, w_attn, w_proj)
    res = run_bass_kernel_spmd(nc, in_maps, list(range(8)))
    return combine(res.results, (B, T, C))
